# revision 13
# baseline (speedup 1.0000x reference)
"""Grouped-experts SwiGLU MLP (DeepseekV3 style) for Trainium2, 8 NeuronCores.

Sharding: expert-parallel. Core e owns expert e's weights and its static
4096-token split. No collectives needed — token routing is the host-side
slice, outputs concatenate back in token order.

Per-core kernel (all matmuls in bf16 with fp32 PSUM accumulation):
  gT[h, t] = wg[d, h].T @ xT[d, t]      (accumulate over 16 d-chunks of 128)
  uT[h, t] = wu[d, h].T @ xT[d, t]
  hT[h, t] = silu(gT) * uT              (ACT silu + DVE mul, stored bf16)
  out[t, d] = hT[h, t].T @ wd[h, d]     (accumulate over 11 h-chunks of 128)

x is fed pre-transposed ([dim, tokens]) per core so the contraction dim sits
on SBUF partitions for both operands; weights are DMA-cast fp32->bf16 on
load and stay resident in SBUF for the whole kernel.
"""

import numpy as np

NUM_EXPERTS = 8
DIM = 2048
HIDDEN = 1408
T_E = 4096  # tokens per expert (static equal splits)

P = 128
TN = 512              # token group width (matmul moving dim)
TND = 512             # mm3 (down-proj) moving dim
NG = T_E // TN        # 8 token groups
DC = DIM // P         # 16 contraction chunks for the up/gate matmuls
HC = HIDDEN // P      # 11 contraction chunks for the down matmul
NDO = DIM // TN       # 4 output-dim blocks of 512

_nc_cache = []


def _build_program(n_reps=1, phase=None):
    import contextlib

    import concourse.mybir as mybir
    import concourse.tile as tile
    from concourse import bacc

    fp32 = mybir.dt.float32
    bf16 = mybir.dt.bfloat16
    AF = mybir.ActivationFunctionType

    nc = bacc.Bacc("TRN2", target_bir_lowering=False, debug=False)

    xT = nc.dram_tensor("xt", [DIM, T_E], fp32, kind="ExternalInput")
    wg = nc.dram_tensor("wg", [DIM, HIDDEN], fp32, kind="ExternalInput")
    wu = nc.dram_tensor("wu", [DIM, HIDDEN], fp32, kind="ExternalInput")
    wd = nc.dram_tensor("wd", [HIDDEN, DIM], fp32, kind="ExternalInput")
    out = nc.dram_tensor("out", [T_E, DIM], fp32, kind="ExternalOutput")

    with tile.TileContext(nc) as tc:
        with (
            tc.tile_pool(name="wpool", bufs=1) as wpool,
            # xt double-buffered: group g+1's 16 cast-DMAs (~28us incl SWDGE
            # emission) hide under mm1/2(g) (~97us) instead of only mm3(g)
            # (~24us at real HW matmul rates) — removes a per-group PE stall.
            tc.tile_pool(name="xpool", bufs=2) as xpool,
            tc.tile_pool(name="hpool", bufs=1) as hpool,
            tc.tile_pool(name="spool", bufs=1) as spool,
            tc.tile_pool(name="opool", bufs=1) as opool,
            tc.tile_pool(name="psum", bufs=2, space="PSUM") as psum_pool,
            # n_reps>1 is the timing-only variant: the whole body runs in a
            # hardware loop so one NEFF exec = n_reps kernel iterations
            # (bench.py slope; cancels the ~130ms axon RPC per call).
            tc.For_i(0, n_reps) if n_reps > 1 else contextlib.nullcontext(),
        ):
            # Resident bf16 weights: [128, chunk, free] with the contraction
            # chunk index as the middle dim. DMA-cast fp32->bf16 (SWDGE).
            wg_sb = wpool.tile([P, DC, HIDDEN], bf16, tag="wg")
            wu_sb = wpool.tile([P, DC, HIDDEN], bf16, tag="wu")
            wd_sb = wpool.tile([P, HC, DIM], bf16, tag="wd")
            # Emission order matters for the single SWDGE queue: the first
            # pg(h0) accumulation needs xt0 + wg cols [0:768); loading wg/wu
            # in h-halves (768|640, hh-block aligned) lets the PE start after
            # ~10.5MB instead of waiting for all of wg (15.7MB). wu's first
            # half follows so pu(h0) isn't starved; wd is only read by the
            # first down-projection (~100us in) so it goes last.
            xt0_sb = xpool.tile([P, DC, TN], bf16, tag="xt")
            H_A = 6 * P  # 768
            for c in range(DC):
                nc.gpsimd.dma_start(out=xt0_sb[:, c, :], in_=xT[c * P:(c + 1) * P, 0:TN])
            for c in range(DC):
                nc.gpsimd.dma_start(out=wg_sb[:, c, 0:H_A], in_=wg[c * P:(c + 1) * P, 0:H_A])
            for c in range(DC):
                nc.gpsimd.dma_start(out=wu_sb[:, c, 0:H_A], in_=wu[c * P:(c + 1) * P, 0:H_A])
            for c in range(DC):
                nc.gpsimd.dma_start(out=wg_sb[:, c, H_A:HIDDEN], in_=wg[c * P:(c + 1) * P, H_A:HIDDEN])
            for c in range(DC):
                nc.gpsimd.dma_start(out=wu_sb[:, c, H_A:HIDDEN], in_=wu[c * P:(c + 1) * P, H_A:HIDDEN])
            for c in range(HC):
                nc.gpsimd.dma_start(out=wd_sb[:, c, :], in_=wd[c * P:(c + 1) * P, :])

            for g in range(NG):
                # xT group [128, 16, 512] bf16, DMA-cast per d-chunk.
                if g == 0:
                    xt_sb = xt0_sb
                else:
                    xt_sb = xpool.tile([P, DC, TN], bf16, tag="xt")
                    for c in range(DC):
                        nc.gpsimd.dma_start(
                            out=xt_sb[:, c, :],
                            in_=xT[c * P:(c + 1) * P, g * TN:(g + 1) * TN],
                        )

                ht_sb = hpool.tile([P, HC, TN], bf16, tag="ht")
                if phase == "mm3":
                    for hh in range(HC):
                        nc.gpsimd.dma_start(
                            out=ht_sb[:, hh, :],
                            in_=xT[hh * P:(hh + 1) * P, 0:TN],
                        )
                for hh in (range(0) if phase == "mm3" else range(HC)):
                    pg = psum_pool.tile([P, TN], fp32, tag="pg")
                    pu = psum_pool.tile([P, TN], fp32, tag="pu")
                    for c in range(DC):
                        nc.tensor.matmul(
                            pg,
                            wg_sb[:, c, hh * P:(hh + 1) * P],
                            xt_sb[:, c, :],
                            start=(c == 0),
                            stop=(c == DC - 1),
                        )
                    for c in range(DC):
                        nc.tensor.matmul(
                            pu,
                            wu_sb[:, c, hh * P:(hh + 1) * P],
                            xt_sb[:, c, :],
                            start=(c == 0),
                            stop=(c == DC - 1),
                        )
                    # silu(g)*u = (g * sigmoid(g)) * u. Each DVE op reads at
                    # most one PSUM operand (HW limit NCC_IBVF027); Silu LUT
                    # isn't in CoreSim so sigmoid+mul keeps this sim-testable.
                    sig = spool.tile([P, TN], fp32, tag="sig")
                    sil = spool.tile([P, TN], fp32, tag="sil")
                    nc.scalar.activation(sig, pg, AF.Sigmoid)
                    nc.vector.tensor_mul(sil, pg, sig)
                    nc.vector.tensor_mul(ht_sb[:, hh, :], sil, pu)

                for tb in range(0 if phase == "mm12" else TN // P):
                    # ot double-buffered so the PSUM->SBUF copies of tb don't
                    # wait on tb-1's out DMA.
                    ot = opool.tile([P, DIM], fp32, tag="ot", bufs=2)
                    # hh-outer so one stationary hT load feeds 4 accumulating
                    # matmuls (one per dout block) -> 4x fewer LDWEIGHTS.
                    # po spans 4 PSUM banks; pg/pu take the other 4.
                    po = psum_pool.tile([P, NDO, TN], fp32, tag="po", bufs=1)
                    for hh in range(HC):
                        for do in range(NDO):
                            nc.tensor.matmul(
                                po[:, do, :],
                                ht_sb[:, hh, tb * P:(tb + 1) * P],
                                wd_sb[:, hh, do * TN:(do + 1) * TN],
                                start=(hh == 0),
                                stop=(hh == HC - 1),
                            )
                    for do in range(NDO):
                        nc.vector.tensor_copy(ot[:, do * TN:(do + 1) * TN], po[:, do, :])
                    t0 = g * TN + tb * P
                    nc.sync.dma_start(out=out[t0:t0 + P, :], in_=ot)

    nc.compile()
    return nc


def _get_program():
    if not _nc_cache:
        _nc_cache.append(_build_program())
    return _nc_cache[0]


def _make_in_maps(inputs):
    x = np.asarray(inputs["x"], dtype=np.float32)
    w_gate = np.asarray(inputs["w_gate"], dtype=np.float32)
    w_up = np.asarray(inputs["w_up"], dtype=np.float32)
    w_down = np.asarray(inputs["w_down"], dtype=np.float32)
    xe = x.reshape(NUM_EXPERTS, T_E, DIM)
    in_maps = []
    for e in range(NUM_EXPERTS):
        in_maps.append(
            {
                "xt": np.ascontiguousarray(xe[e].T),
                "wg": np.ascontiguousarray(w_gate[e]),
                "wu": np.ascontiguousarray(w_up[e]),
                "wd": np.ascontiguousarray(w_down[e]),
            }
        )
    return in_maps


def kernel(x, num_tokens_per_expert, w_gate, w_up, w_down, **_ignored):
    from concourse.bass_utils import run_bass_kernel_spmd

    nc = _get_program()
    in_maps = _make_in_maps(
        {"x": x, "w_gate": w_gate, "w_up": w_up, "w_down": w_down}
    )
    res = run_bass_kernel_spmd(nc, in_maps, core_ids=list(range(NUM_EXPERTS)))
    outs = [np.asarray(r["out"], dtype=np.float32) for r in res.results]
    return np.concatenate(outs, axis=0)

